# revision 20
# baseline (speedup 1.0000x reference)
"""ANT_Linear fused kernel for 8 TRN2 NeuronCores (raw Bass, manual sems).

out = fakequant(x) @ W.T + bias; per-128-group absmax scaling of x snapped to
the 15-level e2m1 ('flint') grid.  Data-parallel over tokens: 2048/core,
16 tiles of [128 tokens, 4096 features].

v2: fp16 quant pipeline + DMA-transpose (no PE transposes, no PSUM staging).

Math (fp16 domain): xs = RN16(x*(6/absmax)), |xs| <= 6.  Branch-free snap via
a magnitude-clamped magic add (int32 ops act on PAIRS of fp16 lanes; pure
bitwise is lane-independent so pairing is safe):
  a  = |xs|                    (int32 view: bits & 0x7FFF7FFF)
  t  = max(a, 1.0) * 512       (float; exact fp16 exponent shift)
  c  = RN16(a + t)             (fp16 output rounding quantizes a at
                                quantum 2^(e-1) of max(a,1): 0.5/1/2)
  y  = c - t                   (exact; y = snap(|xs|) on the e2m1 grid)
  du = y * scale16             (broadcast mult on Pool; unsigned dq)
  s  = bits(xs) & 0x80008000   (sign pairs)
  dq = du | s                  (int32 or; signed dequantized activation)
Then dqT = dma_transpose(dq) -> [s, g, t] layout and out = dqT.T @ W.T + bias
on PE (fp16 matmuls, bias via ones-row matmul).

Per tile [128, 4096] steady state (~15us/tile, PE/DVE-bound):
  DVE : absmax reduce (4.3us) + scale/rr smalls + a/t/c/y/s/or (~10us)
  ACT : xs = x*rr as 32 per-group Copy-with-scale activations (9.3us)
        + PSUM->SBUF out copy + out DMA issue
  Pool: du broadcast mult (8.1us)
  PE  : 64 fp16 matmuls + 2 bias matmuls (14.1us)
  DMA : x-in (5.8) + dq transpose (3.6) + out (1.5)

The sign-or lags one tile behind the y-computation so the Pool mult for tile
i overlaps DVE's front-chain for tile i+1.
"""

import numpy as np

N_CORES = 8
TOK = 4 * 4096
TPC = TOK // N_CORES    # 2048
K = 4096
M = 1024
GS = 128
G = K // GS             # 32
TT = 128
NT = TPC // TT          # 16

NXB = 2                 # x tile buffers
WCH = 4                 # weight DMA chunks (8 k-blocks each)

_CACHE = {}


def _build_bass(nt=NT, debug=False):
    from contextlib import ExitStack

    import concourse.bass as bass
    import concourse.mybir as mybir

    dt = mybir.dt
    alu = mybir.AluOpType
    AF = mybir.ActivationFunctionType

    NTL = nt
    nc = bass.Bass()
    x_d = nc.declare_dram_parameter("x", [NTL * TT, K], dt.float32, isOutput=False)
    wt_d = nc.declare_dram_parameter("wt", [K, M], dt.float16, isOutput=False)
    b_d = nc.declare_dram_parameter("bias", [1, M], dt.float16, isOutput=False)
    out_d = nc.declare_dram_parameter("out", [NTL * TT, M], dt.float32, isOutput=True)
    if debug:
        dbg_xs = nc.declare_dram_parameter("dbg_xs", [TT, K], dt.float16, isOutput=True)
        dbg_y = nc.declare_dram_parameter("dbg_y", [TT, K], dt.float16, isOutput=True)
        dbg_du = nc.declare_dram_parameter("dbg_du", [TT, K], dt.float16, isOutput=True)
        dbg_dq = nc.declare_dram_parameter("dbg_dq", [TT, K], dt.float16, isOutput=True)
        dbg_dqt = nc.declare_dram_parameter("dbg_dqt", [128, G, TT], dt.float16, isOutput=True)
        dbg_rr = nc.declare_dram_parameter("dbg_rr", [TT, G], dt.float32, isOutput=True)
        dbg_sc = nc.declare_dram_parameter("dbg_sc", [TT, G], dt.float32, isOutput=True)

    x_t4 = x_d.rearrange("(n p) (g s) -> n p g s", p=TT, s=GS)   # [16,128,32,128]
    wt_t3 = wt_d.rearrange("(b p) m -> p b m", p=128)            # [128,32,1024]

    ctx = ExitStack()
    with ctx:
        sb = lambda name, shape, d: ctx.enter_context(nc.sbuf_tensor(name, shape, d))
        ps = lambda name, shape, d: ctx.enter_context(nc.psum_tensor(name, shape, d))
        sem = lambda name: ctx.enter_context(nc.semaphore(name))

        wt_sb = sb("wt_sb", [128, G, M], dt.float16)            # 8 MiB resident
        bias_sb = sb("bias_sb", [1, M], dt.float16)
        ones_sb = sb("ones_sb", [1, TT], dt.float16)

        x_sb = [sb(f"x_sb{k}", [TT, G, GS], dt.float32) for k in range(NXB)]
        xs_sb = [sb(f"xs_sb{k}", [TT, K], dt.float16) for k in range(2)]
        a_sb = sb("a_sb", [TT, K], dt.float16)
        t_sb = sb("t_sb", [TT, K], dt.float16)
        y_sb = [sb(f"y_sb{k}", [TT, K], dt.float16) for k in range(2)]
        du_sb = sb("du_sb", [TT, K], dt.float16)
        dq_sb = [sb(f"dq_sb{k}", [TT, K], dt.float16) for k in range(2)]
        # dense [s, g, t]: the xbar transpose writes group-pitch-128 packed
        dqt_sb = [sb(f"dqt_sb{k}", [128, G, TT], dt.float16) for k in range(2)]
        o_sb = [sb(f"o_sb{k}", [TT, M], dt.float32) for k in range(2)]
        dbgd_sb = sb("dbgd_sb", [128, G * TT], dt.float16) if debug else None
        amax_sb = sb("amax_sb", [TT, G], dt.float32)
        sc_sb = [sb(f"sc_sb{k}", [TT, G], dt.float32) for k in range(8)]
        rr_sb = [sb(f"rr_sb{k}", [TT, G], dt.float32) for k in range(8)]

        pout_ps = [ps(f"pout_ps{k}", [TT, M], dt.float32) for k in range(2)]

        sC = sem("sC")     # bias DMA done
        sV = sem("sV")     # vector consts (ones) ready
        sX = sem("sX")     # x DMA in (+16/tile)
        sWT = sem("sWT")   # weight chunk in (+16/chunk)
        sRD = sem("sRD")   # DVE reduce done (x half-free)
        sRR = sem("sRR")   # rr/scale16 ready
        sXS = sem("sXS")   # ACT xs done (all 32 groups)
        sY = sem("sY")     # DVE y done (front chain)
        sDU = sem("sDU")   # Pool du done
        sDQ = sem("sDQ")   # DVE sign-or done (dq final)
        sTP = sem("sTP")   # dq transpose DMA done (+16)
        sMM = sem("sMM")   # PE matmuls done (dqt free, pout full)
        sOC = sem("sOC")   # ACT out copy done (pout free)
        sOD = sem("sOD")   # out DMA done (o_sb free, +16)

        xs_i32 = [xs_sb[k].bitcast(dt.int32) for k in range(2)]
        y3 = [y_sb[k].rearrange("p (g s) -> p g s", s=GS) for k in range(2)]
        du3 = du_sb.rearrange("p (g s) -> p g s", s=GS)
        a_i32 = a_sb.bitcast(dt.int32)
        du_i32 = du_sb.bitcast(dt.int32)
        dq_i32 = [dq_sb[k].bitcast(dt.int32) for k in range(2)]

        def tp(eng, j):
            if j < 0 or j >= NTL:
                return
            eng.wait_ge(sDQ, j + 1)            # dq(j) ready
            if j >= 2:
                eng.wait_ge(sMM, j - 1)        # dqt buf free
            eng.dma_start(
                out=dqt_sb[j % 2][:, :, :],
                in_=dq_sb[j % 2][:, :],
                transpose=True,
            ).then_inc(sTP, 16)

        with nc.Block() as block:

            @block.sync
            def _(eng):
                # startup: x0 first, then weights in 4 chunks interleaved
                # with the next x tiles, so tile-0 compute overlaps wt load
                eng.dma_start(out=x_sb[0][:, :, :], in_=x_t4[0]).then_inc(sX, 16)
                eng.dma_start(
                    out=wt_sb[:, 0:8, :], in_=wt_t3[:, 0:8, :]
                ).then_inc(sWT, 16)
                if NTL > 1:
                    eng.dma_start(
                        out=x_sb[1][:, :, :], in_=x_t4[1]
                    ).then_inc(sX, 16)
                for c in range(1, WCH):
                    eng.dma_start(
                        out=wt_sb[:, 8 * c:8 * (c + 1), :],
                        in_=wt_t3[:, 8 * c:8 * (c + 1), :],
                    ).then_inc(sWT, 16)
                eng.dma_start(out=bias_sb[:, :], in_=b_d[:, :]).then_inc(sC, 16)
                # steady state: x tile loads + dq transposes, interleaved
                for i in range(2, NTL):
                    eng.wait_ge(sRD, i - 1)        # x buf: reduce(i-2) done
                    eng.wait_ge(sXS, i - 1)        # x buf: xs(i-2) done
                    eng.dma_start(
                        out=x_sb[i % NXB][:, :, :], in_=x_t4[i]
                    ).then_inc(sX, 16)
                    tp(eng, i - 3)
                tp(eng, NTL - 3)
                tp(eng, NTL - 2)
                tp(eng, NTL - 1)
                if debug:
                    eng.wait_ge(sRR, 1)
                    eng.dma_start(out=dbg_rr[:, :], in_=rr_sb[0][:, :]).then_inc(sC, 16)
                    eng.dma_start(out=dbg_sc[:, :], in_=sc_sb[0][:, :]).then_inc(sC, 16)
                    eng.wait_ge(sXS, 1)
                    eng.dma_start(out=dbg_xs[:, :], in_=xs_sb[0][:, :]).then_inc(sC, 16)
                    eng.wait_ge(sY, 1)
                    eng.dma_start(out=dbg_y[:, :], in_=y_sb[0][:, :]).then_inc(sC, 16)
                    eng.wait_ge(sDU, 1)
                    eng.dma_start(out=dbg_du[:, :], in_=du_sb[:, :]).then_inc(sC, 16)
                    eng.wait_ge(sDQ, 1)
                    eng.dma_start(out=dbg_dq[:, :], in_=dq_sb[0][:, :]).then_inc(sC, 16)
                    pass

            @block.vector
            def _(eng):
                nc.vector.memset(ones_sb[:, :], 1.0)
                nc.vector.drain().then_inc(sV, 1)

                def stats(j):
                    if j >= NTL:
                        return
                    eng.wait_ge(sX, 16 * (j + 1))
                    nc.vector.tensor_reduce(
                        out=amax_sb[:, :], in_=x_sb[j % NXB][:, :, :],
                        axis=mybir.AxisListType.X, op=alu.max,
                        apply_absolute_value=True,
                    )
                    nc.vector.drain().then_inc(sRD, 1)
                    # scale = max(amax/6, tiny); rr = 1/scale
                    nc.vector.tensor_scalar(
                        out=sc_sb[j % 8][:, :], in0=amax_sb[:, :],
                        scalar1=1.0 / 6.0, scalar2=1e-30,
                        op0=alu.mult, op1=alu.max,
                    )
                    nc.vector.drain()
                    nc.vector.reciprocal(
                        out=rr_sb[j % 8][:, :], in_=sc_sb[j % 8][:, :]
                    )
                    nc.vector.drain().then_inc(sRR, 1)

                def front(j):
                    if j < 0 or j >= NTL:
                        return
                    eng.wait_ge(sXS, j + 1)
                    # a = |xs| (int32 pair bitwise)
                    nc.vector.tensor_scalar(
                        out=a_i32[:, :], in0=xs_i32[j % 2][:, :],
                        scalar1=0x7FFF7FFF, scalar2=None, op0=alu.bitwise_and,
                    )
                    nc.vector.drain()
                    # t = max(a, 1) * 512  (float, exact exponent shift)
                    nc.vector.tensor_scalar(
                        out=t_sb[:, :], in0=a_sb[:, :],
                        scalar1=1.0, scalar2=512.0,
                        op0=alu.max, op1=alu.mult,
                    )
                    nc.vector.drain()
                    # c = RN16(a + t): rounds a to the grid quantum
                    if j >= 2:
                        eng.wait_ge(sDU, j - 1)    # y buf free (du(j-2) done)
                    nc.vector.tensor_tensor(
                        out=y_sb[j % 2][:, :], in0=a_sb[:, :], in1=t_sb[:, :],
                        op=alu.add,
                    )
                    nc.vector.drain()
                    # y = c - t  (exact; in place)
                    nc.vector.tensor_tensor(
                        out=y_sb[j % 2][:, :], in0=y_sb[j % 2][:, :],
                        in1=t_sb[:, :], op=alu.subtract,
                    )
                    nc.vector.drain().then_inc(sY, 1)
                    # s = sign pairs of xs, staged directly into the dq buffer
                    if j >= 2:
                        eng.wait_ge(sTP, 16 * (j - 1))  # dq buf free
                    nc.vector.tensor_scalar(
                        out=dq_i32[j % 2][:, :], in0=xs_i32[j % 2][:, :],
                        scalar1=-2147450880, scalar2=None, op0=alu.bitwise_and,
                    )
                    nc.vector.drain()

                def sign_or(j):
                    if j < 0 or j >= NTL:
                        return
                    eng.wait_ge(sDU, j + 1)        # du(j) ready
                    nc.vector.tensor_tensor(
                        out=dq_i32[j % 2][:, :], in0=du_i32[:, :],
                        in1=dq_i32[j % 2][:, :], op=alu.bitwise_or,
                    )
                    nc.vector.drain().then_inc(sDQ, 1)

                stats(0)
                stats(1)
                for i in range(NTL + 1):
                    stats(i + 2)
                    front(i)
                    sign_or(i - 1)

            @block.gpsimd
            def _(eng):
                for i in range(NTL):
                    eng.wait_ge(sX, 16 * (i + 1))
                    eng.wait_ge(sRR, i + 1)
                    if i >= 2:
                        eng.wait_ge(sDQ, i - 1)    # xs buf: chain(i-2) done
                    r_b = rr_sb[i % 8][:, :].unsqueeze(2).broadcast_to(
                        (TT, G, GS)
                    )
                    nc.gpsimd.tensor_tensor(
                        out=xs_sb[i % 2].rearrange(
                            "p (g s) -> p g s", s=GS)[:, :, :],
                        in0=x_sb[i % NXB][:, :, :],
                        in1=r_b, op=alu.mult,
                    )
                    nc.gpsimd.drain().then_inc(sXS, 1)

            @block.scalar
            def _(eng):
                def mk_du(j):
                    eng.wait_ge(sY, j + 1)
                    if j >= 1:
                        eng.wait_ge(sDQ, j)        # du buf free (or(j-1) done)
                    for g in range(G):
                        nc.scalar.activation(
                            out=du3[:, g, :], in_=y3[j % 2][:, g, :],
                            func=AF.Copy, scale=sc_sb[j % 8][:, g:g + 1],
                        )
                    nc.scalar.drain().then_inc(sDU, 1)

                def out_copy(j):
                    if j < 0 or j >= NTL:
                        return
                    eng.wait_ge(sMM, j + 1)
                    if j >= 2:
                        eng.wait_ge(sOD, 16 * (j - 1))  # o_sb free
                    nc.scalar.activation(
                        out=o_sb[j % 2][:, :], in_=pout_ps[j % 2][:, :],
                        func=AF.Copy,
                    )
                    nc.scalar.drain().then_inc(sOC, 1)
                    eng.dma_start(
                        out=out_d[j * TT:(j + 1) * TT, :], in_=o_sb[j % 2][:, :]
                    ).then_inc(sOD, 16)

                for i in range(NTL):
                    mk_du(i)
                    out_copy(i - 3)
                if debug:
                    eng.wait_ge(sTP, 16)
                    nc.scalar.activation(
                        out=dbgd_sb.rearrange("p (g t) -> p g t", t=TT)[:, :, :],
                        in_=dqt_sb[0][:, :, :], func=AF.Copy,
                    )
                    nc.scalar.drain()
                    eng.dma_start(
                        out=dbg_dqt[:, :, :],
                        in_=dbgd_sb.rearrange("p (g t) -> p g t", t=TT)[:, :, :],
                    ).then_inc(sC, 16)
                out_copy(NTL - 3)
                out_copy(NTL - 2)
                out_copy(NTL - 1)

            @block.tensor
            def _(eng):
                eng.wait_ge(sC, 16)
                eng.wait_ge(sV, 1)
                for i in range(NTL):
                    eng.wait_ge(sTP, 16 * (i + 1))
                    if i >= 2:
                        eng.wait_ge(sOC, i - 1)    # pout buf free
                    for b in range(G):
                        if i == 0 and b % 8 == 0:
                            eng.wait_ge(sWT, 16 * (b // 8 + 1))
                        for hf in range(2):
                            nc.tensor.matmul(
                                pout_ps[i % 2][:, hf * 512:(hf + 1) * 512],
                                lhsT=dqt_sb[i % 2][:, b, :],
                                rhs=wt_sb[:, b, hf * 512:(hf + 1) * 512],
                                start=(b == 0),
                                stop=False,
                            )
                    for hf in range(2):
                        ins = nc.tensor.matmul(
                            pout_ps[i % 2][:, hf * 512:(hf + 1) * 512],
                            lhsT=ones_sb[:, :],
                            rhs=bias_sb[:, hf * 512:(hf + 1) * 512],
                            start=False,
                            stop=True,
                        )
                        if hf == 1:
                            ins.then_inc(sMM, 1)

    return nc


def _get_nc():
    if "nc" not in _CACHE:
        _CACHE["nc"] = _build_bass()
    return _CACHE["nc"]


def make_in_maps(x, weight, bias):
    x2 = np.ascontiguousarray(np.asarray(x, dtype=np.float32).reshape(TOK, K))
    wt = np.ascontiguousarray(np.asarray(weight, dtype=np.float32).T).astype(
        np.float16
    )
    bias_h = np.asarray(bias, dtype=np.float32).reshape(1, M).astype(np.float16)
    return [
        {"x": x2[i * TPC:(i + 1) * TPC], "wt": wt, "bias": bias_h}
        for i in range(N_CORES)
    ]


def kernel(x, weight, bias, grid=None, **_ignored):
    from concourse.bass_utils import run_bass_kernel_spmd

    nc = _get_nc()
    in_maps = make_in_maps(x, weight, bias)
    res = run_bass_kernel_spmd(nc, in_maps, core_ids=list(range(N_CORES)))
    out = np.concatenate([res.results[i]["out"] for i in range(N_CORES)], axis=0)
    return out.reshape(4, 4096, M).astype(np.float32)
